# revision 1
# baseline (speedup 1.0000x reference)
"""Trainium2 Bass kernel for CGPCoupler gather-multiply-scatter (segment reduce).

Computation (reference):
    out_tilde = x1[:, r1] * x2[:, r2] * cg[None, :]        # [B, K]
    out = zeros([B, out_dim]).at[:, ro].add(out_tilde)

Structure exploited: the CG coupler's index tables consist of K/32 runs of 32
consecutive 32-aligned indices with a constant coefficient per run, i.e. T
block-triples:
    out[:, o*32:+32] += c_t * x1[:, a*32:+32] * x2[:, b*32:+32]

Device mapping:
  * data-parallel over batch: 8 cores x 256 rows.
  * SBUF layout: batch-on-partition, 2 batch subtiles packed into the free
    dim so a 32-col block is one contiguous 64-wide column group
    (col = blk*64 + s*32 + c).  HBM stores stay fully contiguous per row.
  * per triple, one fused DVE op (custom TENSOR_TENSOR_REDUCE:
    out = in0*in1*s1).  Triples are grouped into affine chains
    (a0+k*da, b0+k*db, dst0+k*dd) with equal coefficient -> one rank-3
    strided-AP instruction per chain.
  * first contribution per out block writes directly; later contributions
    go to rank-class tmp arrays and are folded in with run-grouped adds.
"""

import dataclasses
import numpy as np
from collections import Counter, defaultdict

N_CORES = 8


# ----------------------------------------------------------------- planning
def _extract_triples(r1, r2, ro, cg):
    """Detect 32-run structure; return (a, b, o, c) per 32-block triple or None."""
    K = cg.shape[0]
    if K % 32 != 0:
        return None
    T = K // 32
    lane = np.arange(32, dtype=np.int64)
    for arr in (r1, r2, ro):
        v = arr.astype(np.int64).reshape(T, 32)
        if not np.all(v == v[:, :1] + lane):
            return None
        if np.any(v[:, 0] % 32):
            return None
    cgv = cg.reshape(T, 32)
    if not np.all(cgv == cgv[:, :1]):
        return None
    a = (r1.astype(np.int64)[::32] // 32).astype(int)
    b = (r2.astype(np.int64)[::32] // 32).astype(int)
    o = (ro.astype(np.int64)[::32] // 32).astype(int)
    c = cgv[:, 0].astype(np.float64)
    return a, b, o, c


def _greedy_chains(pts):
    """Cover point set (a,b,dst) by affine chains; returns [(p0, delta, r)]."""
    pts = set(pts)
    groups = []
    while pts:
        pl = sorted(pts)
        if len(pl) == 1:
            groups.append((pl[0], (0, 0, 0), 1))
            pts.remove(pl[0])
            break
        best = None
        for p in pl:
            for q in pl:
                if p >= q:
                    continue
                d = (q[0] - p[0], q[1] - p[1], q[2] - p[2])
                s = p
                while (s[0] - d[0], s[1] - d[1], s[2] - d[2]) in pts:
                    s = (s[0] - d[0], s[1] - d[1], s[2] - d[2])
                chain = [s]
                nxt = (s[0] + d[0], s[1] + d[1], s[2] + d[2])
                while nxt in pts:
                    chain.append(nxt)
                    nxt = (nxt[0] + d[0], nxt[1] + d[1], nxt[2] + d[2])
                if best is None or len(chain) > len(best[0]):
                    best = (chain, d)
        chain, d = best
        if d[2] < 0:  # canonicalize: dst stride positive
            chain = chain[::-1]
            d = (-d[0], -d[1], -d[2])
        groups.append((chain[0], d, len(chain)))
        for p in set(chain):
            pts.discard(p)
    return groups


def _greedy_chains2(pts):
    """Cover 2-D point set by affine chains; returns [(p0, (d0,d1), n)]."""
    pts = set(pts)
    groups = []
    while pts:
        pl = sorted(pts)
        if len(pl) == 1:
            groups.append((pl[0], (0, 0), 1))
            pts.remove(pl[0])
            break
        best = None
        for p in pl:
            for q in pl:
                if p >= q:
                    continue
                d = (q[0] - p[0], q[1] - p[1])
                s = p
                while (s[0] - d[0], s[1] - d[1]) in pts:
                    s = (s[0] - d[0], s[1] - d[1])
                chain = [s]
                nxt = (s[0] + d[0], s[1] + d[1])
                while nxt in pts:
                    chain.append(nxt)
                    nxt = (nxt[0] + d[0], nxt[1] + d[1])
                if best is None or len(chain) > len(best[0]):
                    best = (chain, d)
        chain, d = best
        groups.append((chain[0], d, len(chain)))
        for p in set(chain):
            pts.discard(p)
    return groups


def _group_add_runs(add_runs, band=1):
    """Group equal-length add runs into affine (o0,j0) chains within o-bands.
    Returns [(rank, o0, j0, do, dj, n, L)]."""
    from collections import defaultdict
    out = []
    byk = defaultdict(list)
    for rk, o0, j0, r in add_runs:
        byk[(rk, r, o0 // band)].append((o0, j0))
    for (rk, L, _bnd), pts in sorted(byk.items()):
        for (o0, j0), (do, dj), n in _greedy_chains2(pts):
            out.append((rk, o0, j0, do, dj, n, L))
    return out


def _make_plan(a, b, o, c):
    T = len(a)
    order = np.lexsort((np.arange(T), o))
    cnt = Counter()
    rank = np.zeros(T, int)
    for i in order:
        cnt[o[i]] += 1
        rank[i] = cnt[o[i]]
    max_rank = int(rank.max()) if T else 0

    cr = np.round(c, 12)
    ttr_chains = []          # (rank, c, (a0,b0,d0), (da,db,dd), r)
    tmp_sizes = {}
    add_runs = []            # (rank, o0, j0, r)
    for rk in range(1, max_rank + 1):
        idxs = [i for i in range(T) if rank[i] == rk]
        idxs.sort(key=lambda i: o[i])
        if rk > 1:
            tmp_sizes[rk] = len(idxs)
            jof = {i: j for j, i in enumerate(idxs)}
            # add runs: consecutive o (and hence consecutive j)
            start = 0
            for k in range(1, len(idxs) + 1):
                if k == len(idxs) or o[idxs[k]] != o[idxs[k - 1]] + 1:
                    add_runs.append((rk, int(o[idxs[start]]), start, k - start))
                    start = k
        classes = defaultdict(list)
        for i in idxs:
            classes[cr[i]].append(i)
        for cv, ii in classes.items():
            if rk == 1:
                pts = [(int(a[i]), int(b[i]), int(o[i])) for i in ii]
            else:
                pts = [(int(a[i]), int(b[i]), int(jof[i])) for i in ii]
            for p0, d, r in _greedy_chains(pts):
                ttr_chains.append((rk, float(cv), p0, d, r))
    return ttr_chains, tmp_sizes, add_runs


def _numpy_fallback(x1, x2, cg_tilde, repids_in1, repids_in2, repids_out, out_dim):
    out_tilde = x1[:, repids_in1] * x2[:, repids_in2] * cg_tilde[None, :]
    out = np.zeros((x1.shape[0], int(out_dim)), dtype=x1.dtype)
    np.add.at(out, (slice(None), repids_out), out_tilde)
    return out


# ----------------------------------------------------------------- bass build
_nc_cache = {}


def _slice_blocks(ap, start, step, r, P=128):
    """[P, nblk, 64] AP -> [P, r, 64] starting at `start` with block stride `step`."""
    if r == 1:
        return ap[:, start:start + 1, :]
    if step == 0:
        return ap[:, start:start + 1, :].to_broadcast([P, r, 64])
    if step > 0:
        return ap[:, start: start + step * (r - 1) + 1: step, :]
    stop = start + step * (r - 1) - 1
    return ap[:, start: (stop if stop >= 0 else None): step, :]


def _build_nc(ttr_chains, tmp_sizes, add_runs, in_dim, out_dim, b_core,
              n_store_chunks=8, no_broadcast=False):
    import concourse.bacc as bacc
    from concourse import mybir
    from concourse.tile import TileContext
    from concourse.dve_ops import TENSOR_TENSOR_REDUCE

    f32 = mybir.dt.float32
    bf16 = mybir.dt.bfloat16
    S = b_core // 128
    assert S == 2, "layout assumes 2 batch subtiles per core"
    n_ablk = in_dim // 32
    n_oblk = out_dim // 32

    nc = bacc.Bacc("TRN2", target_bir_lowering=False)
    x1 = nc.dram_tensor("x1", [b_core, in_dim], f32, kind="ExternalInput")
    x2 = nc.dram_tensor("x2", [b_core, in_dim], f32, kind="ExternalInput")
    y = nc.dram_tensor("y", [b_core, out_dim], f32, kind="ExternalOutput")

    # tapered chunks: big early windows, small tail to shorten the drain
    w = np.array([40, 40, 40, 36, 32, 28, 24, 16, 16, 8], dtype=int)
    w = (w * n_oblk / w.sum()).astype(int)
    w[0] += n_oblk - w.sum()
    chunk_edges = np.concatenate([[0], np.cumsum(w)])
    n_store_chunks = len(w)

    with TileContext(nc) as tc:
        with (
            tc.tile_pool(name="pin", bufs=1) as pin,
            tc.tile_pool(name="pout", bufs=1) as pout,
            tc.tile_pool(name="ptmp", bufs=1) as ptmp,
            tc.tile_pool(name="pstg", bufs=3) as pstg,
            tc.tile_pool(name="pjunk", bufs=32) as pjunk,
        ):
            # inputs held in bf16: the ACT reorder copies below cast
            # fp32->bf16; DVE TTR chains then stream 2 elems/cycle/lane
            x1t = pin.tile([128, n_ablk * 64], bf16, tag="x1t")
            x2t = pin.tile([128, n_ablk * 64], bf16, tag="x2t")
            # contiguous loads into s-major staging, ACT reorders into the
            # blocked layout (SBUF col f*64 + s*32 + c <- HBM row s*128+p,
            # col f*32+c)
            copyf = mybir.ActivationFunctionType.Copy
            for xt, xd, nm in ((x1t, x1, "l1"), (x2t, x2, "l2")):
                lstg = pstg.tile([128, S, in_dim], f32, tag="stg")
                xv = xt[:].rearrange("p (f s c) -> p s f c", s=S, c=32)
                for s in range(S):
                    nc.sync.dma_start(out=lstg[:, s, :],
                                      in_=xd[s * 128:(s + 1) * 128, :])
                    nc.scalar.activation(
                        out=xv[:, s],
                        in_=lstg[:, s, :].rearrange("p (f c) -> p f c", c=32),
                        func=copyf,
                    )
            outt = pout.tile([128, n_oblk * 64], f32, tag="outt")

            x1b = x1t[:].rearrange("p (f v) -> p f v", v=64)
            x2b = x2t[:].rearrange("p (f v) -> p f v", v=64)
            outb = outt[:].rearrange("p (f v) -> p f v", v=64)
            tmps = {}
            tmps_flat = {}
            for rk, sz in tmp_sizes.items():
                t = ptmp.tile([128, sz * 64], f32, tag=f"tmp{rk}")
                tmps_flat[rk] = t
                tmps[rk] = t[:].rearrange("p (f v) -> p f v", v=64)

            # Order compute ops by the MIN output block they touch, then emit
            # each store window as soon as the last op touching it has been
            # emitted.  (Ordering by min keeps long-span chains from delaying
            # early windows; correctness only needs every touching op to
            # precede the window's reorder/store.)
            o_of_tmp = {}
            for rk, o0, j0, r in add_runs:
                for k in range(r):
                    o_of_tmp[(rk, j0 + k)] = o0 + k
            work = []  # (minkey, seq, kind, payload, touched_blocks)
            seq = 0
            for rk, cv, p0, d, r in ttr_chains:
                dsts = [p0[2] + d[2] * k for k in range(r)]
                if rk == 1:
                    touched = dsts
                else:
                    touched = [o_of_tmp[(rk, j)] for j in dsts]
                work.append((min(touched), seq, "ttr", (rk, cv, p0, d, r),
                             touched))
                seq += 1
            add_groups = _group_add_runs(add_runs)
            if no_broadcast:
                add_groups = [(rk, o0 + k * do, j0 + k * dj, 0, 0, 1, L)
                              for (rk, o0, j0, do, dj, n, L) in add_groups
                              for k in range(n)]
            for rk, o0, j0, do, dj, n, L in add_groups:
                touched = [o0 + k * do + i for k in range(n) for i in range(L)]
                # keyed by MAX block: every producer chain of a block k in the
                # run has key = min(chain blocks) <= k <= max and earlier seq,
                # so all tmp/rank-1 writes precede this add
                work.append((max(touched), seq, "add",
                             (rk, o0, j0, do, dj, n, L), touched))
                seq += 1
            work.sort(key=lambda t: (t[0], t[1]))
            # window-close position: index of last op touching each window
            close_at = {}
            for idx, (_, _, kind, pl, touched) in enumerate(work):
                for ci in range(n_store_chunks):
                    o_lo, o_hi = int(chunk_edges[ci]), int(chunk_edges[ci + 1])
                    if any(o_lo <= t < o_hi for t in touched):
                        close_at[ci] = idx
            fixed = []
            for idx, (key, sq, kind, pl, _) in enumerate(work):
                fixed.append((kind, pl))
                for ci in range(n_store_chunks):
                    if close_at.get(ci) == idx:
                        o_lo, o_hi = int(chunk_edges[ci]), int(chunk_edges[ci + 1])
                        if o_hi > o_lo:
                            fixed.append(("store", (o_lo, o_hi)))

            outv = outt[:].rearrange("p (f s c) -> p s f c", s=S, c=32)
            for kind, pl in fixed:
                if kind == "ttr":
                    rk, cv, p0, d, r = pl
                    a0, b0, d0 = p0
                    da, db, dd = d
                    dstb = outb if rk == 1 else tmps[rk]
                    pieces = [(a0, b0, d0, da, db, dd, r)]
                    if no_broadcast and r > 1:
                        # interp-only mode: the CoreSim custom-DVE reference
                        # can't handle mixed merged/strided AP shapes
                        pieces = [(a0 + k * da, b0 + k * db, d0 + k * dd,
                                   0, 0, 0, 1) for k in range(r)]
                    for (ca, cb, cd, xda, xdb, xdd, cr) in pieces:
                        junk = pjunk.tile([128, 1], f32, tag="junk")
                        nc.vector._custom_dve(
                            TENSOR_TENSOR_REDUCE,
                            out=_slice_blocks(dstb, cd, xdd, cr),
                            in0=_slice_blocks(x1b, ca, xda, cr),
                            in1=_slice_blocks(x2b, cb, xdb, cr),
                            s0=0.0, s1=float(cv), accum_out=junk[:],
                        )
                elif kind == "add":
                    rk, o0, j0, do, dj, n, L = pl
                    if n == 1:
                        dst = outt[:, o0 * 64:(o0 + L) * 64]
                        src = tmps_flat[rk][:, j0 * 64:(j0 + L) * 64]
                    else:
                        oc = n_oblk * 64
                        tc_ = tmp_sizes[rk] * 64
                        dst = dataclasses.replace(
                            outt[:], ap=[[oc, 128], [do * 64, n], [1, L * 64]],
                            offset=o0 * 64)
                        src = dataclasses.replace(
                            tmps_flat[rk][:],
                            ap=[[tc_, 128], [dj * 64, n], [1, L * 64]],
                            offset=j0 * 64)
                    # late adds run on DVE: it is idle once its chains finish,
                    # while gpsimd still drains its backlog
                    eng = nc.vector if o0 >= 230 else nc.gpsimd
                    eng.tensor_add(out=dst, in0=dst, in1=src)
                else:  # store: ACT reorders blocked (o,s,c) -> s-major staging
                    o_lo, o_hi = pl
                    w = o_hi - o_lo
                    stg = pstg.tile([128, S, w * 32], f32, tag="stg")
                    for s in range(S):
                        nc.scalar.activation(
                            out=stg[:, s, :], in_=outv[:, s, o_lo:o_hi, :],
                            func=copyf,
                        )
                        nc.sync.dma_start(
                            out=y[s * 128:(s + 1) * 128, o_lo * 32:o_hi * 32],
                            in_=stg[:, s, :],
                        )
    nc.finalize()
    return nc


def _get_nc(triples, in_dim, out_dim, b_core, no_broadcast=False):
    a, b, o, c = triples
    key = hash((in_dim, out_dim, b_core, no_broadcast, tuple(a), tuple(b),
                tuple(o), tuple(np.asarray(c).tolist())))
    if key not in _nc_cache:
        ttr_chains, tmp_sizes, add_runs = _make_plan(a, b, o, c)
        _nc_cache[key] = _build_nc(ttr_chains, tmp_sizes, add_runs,
                                   in_dim, out_dim, b_core,
                                   no_broadcast=no_broadcast)
    return _nc_cache[key]


# ----------------------------------------------------------------- entry point
def kernel(x1, x2, cg_tilde, repids_in1, repids_in2, repids_out, out_dim):
    x1 = np.ascontiguousarray(np.asarray(x1, dtype=np.float32))
    x2 = np.ascontiguousarray(np.asarray(x2, dtype=np.float32))
    cg = np.asarray(cg_tilde, dtype=np.float32)
    r1 = np.asarray(repids_in1)
    r2 = np.asarray(repids_in2)
    ro = np.asarray(repids_out)
    odim = int(np.asarray(out_dim))

    B, in_dim = x1.shape
    triples = _extract_triples(r1, r2, ro, cg)
    usable = (
        triples is not None and B % N_CORES == 0
        and (B // N_CORES) == 256 and odim % 32 == 0 and in_dim % 32 == 0
    )
    if not usable:
        return _numpy_fallback(x1, x2, cg, r1, r2, ro, odim)

    from concourse.bass_utils import run_bass_kernel_spmd

    b_core = B // N_CORES
    nc = _get_nc(triples, in_dim, odim, b_core)

    in_maps = [
        {"x1": x1[i * b_core:(i + 1) * b_core],
         "x2": x2[i * b_core:(i + 1) * b_core]}
        for i in range(N_CORES)
    ]
    res = run_bass_kernel_spmd(nc, in_maps, core_ids=list(range(N_CORES)))
    out = np.empty((B, odim), dtype=np.float32)
    for i in range(N_CORES):
        out[i * b_core:(i + 1) * b_core] = res.results[i]["y"]
    return out



# revision 13
# speedup vs baseline: 1.3203x; 1.3203x over previous
"""Trainium2 Bass kernel for CGPCoupler gather-multiply-scatter (segment reduce).

Computation (reference):
    out_tilde = x1[:, r1] * x2[:, r2] * cg[None, :]        # [B, K]
    out = zeros([B, out_dim]).at[:, ro].add(out_tilde)

Structure exploited: the index tables are runs of 32 consecutive aligned
indices with constant coefficient -> T block-triples
    out[:, o*32:+32] += c_t * x1[:, a*32:+32] * x2[:, b*32:+32]
with only ~280 DISTINCT products (a,b), each feeding 1-4 outputs, and all
rank-1 outputs having c = +-1.

Device mapping: data-parallel over batch (8 cores x 256 rows); SBUF layout
batch-on-partition with 2 row-subtiles packed into 64-wide column groups.

Pipeline (all intermediates fp16 for the DVE 2x/4x fast paths):
  1. DMA loads f32 -> ACT cast+reorder to fp16 blocked-64 (x1t, x2t);
     DVE negated copy x1n = -x1t (TensorScalar, 4x mode).
  2. products: TensorTensor fp16 (2x) -> s2[o] directly (rank-1, c=+-1)
     or shared z slots.
  3. scaled copies w = ratio * z (TensorScalar 4x) for cross-coefficient
     contributions; add/sub trees accumulate into s2[o] (TensorTensor 2x,
     in-place RMW); per-output pivot scale applied in-place (TensorScalar 4x).
  4. per store window: fused convert+reorder s2 fp16 -> f32 s-major staging
     (TensorCopy 2x / ACT activation / Pool copy), then chunked DMA stores.
All ops are emitted as 2-level affine grids (rank-4 APs, 64-packed last dim)
to amortize per-instruction overheads; work is split DVE/ACT/Pool to keep
every engine under the ~31us DMA streaming floor.
"""

import dataclasses
import numpy as np
from collections import defaultdict

N_CORES = 8

WIN_EDGES = [0, 16, 44, 72, 100, 128, 156, 184, 212, 248, 280]
# convert engine per window: v=DVE, a=ACT, p=Pool
CONV_ENG = ["a", "a", "a", "a", "a", "a", "a", "a", "v", "v"]
POOL_CLASS_BLOCKS = 84   # add blocks moved to Pool via whole-class offload
POOL_CLASS_MINWIN = 3     # only classes whose windows all lie in
POOL_CLASS_MAXWIN = 8     # [POOL_CLASS_MINWIN, POOL_CLASS_MAXWIN]


# ----------------------------------------------------------------- triples
def _extract_triples(r1, r2, ro, cg):
    """Detect 32-run structure; return (a, b, o, c) per 32-block triple or None."""
    K = cg.shape[0]
    if K % 32 != 0:
        return None
    T = K // 32
    lane = np.arange(32, dtype=np.int64)
    for arr in (r1, r2, ro):
        v = arr.astype(np.int64).reshape(T, 32)
        if not np.all(v == v[:, :1] + lane):
            return None
        if np.any(v[:, 0] % 32):
            return None
    cgv = cg.reshape(T, 32)
    if not np.all(cgv == cgv[:, :1]):
        return None
    a = (r1.astype(np.int64)[::32] // 32).astype(int)
    b = (r2.astype(np.int64)[::32] // 32).astype(int)
    o = (ro.astype(np.int64)[::32] // 32).astype(int)
    c = cgv[:, 0].astype(np.float64)
    return a, b, o, c


def _numpy_fallback(x1, x2, cg_tilde, repids_in1, repids_in2, repids_out, out_dim):
    out_tilde = x1[:, repids_in1] * x2[:, repids_in2] * cg_tilde[None, :]
    out = np.zeros((x1.shape[0], int(out_dim)), dtype=x1.dtype)
    np.add.at(out, (slice(None), repids_out), out_tilde)
    return out


# ----------------------------------------------------------------- planning
def _greedy_chains_nd(pts):
    """Cover n-D integer points by affine chains [(p0, delta, r)]."""
    pts = set(pts)
    dim = len(next(iter(pts))) if pts else 0
    groups = []
    while pts:
        pl = sorted(pts)
        if len(pl) == 1:
            groups.append((pl[0], tuple([0] * dim), 1))
            break
        best = None
        for i, p in enumerate(pl):
            for q in pl[i + 1:]:
                d = tuple(q[k] - p[k] for k in range(dim))
                s = p
                prev = tuple(s[k] - d[k] for k in range(dim))
                while prev in pts:
                    s = prev
                    prev = tuple(s[k] - d[k] for k in range(dim))
                chain = [s]
                nxt = tuple(s[k] + d[k] for k in range(dim))
                while nxt in pts:
                    chain.append(nxt)
                    nxt = tuple(nxt[k] + d[k] for k in range(dim))
                if best is None or len(chain) > len(best[0]):
                    best = (chain, d)
                if best is not None and len(best[0]) >= len(pl):
                    break
            else:
                continue
            break
        chain, d = best
        groups.append((chain[0], d, len(chain)))
        for p in chain:
            pts.discard(p)
        if len(chain) == 1 and len(pts) == len(pl) - 1:
            for p in sorted(pts):
                groups.append((p, tuple([0] * dim), 1))
            pts.clear()
    return groups


def _grid_cover(pts):
    """Cover N-D points by 2-level affine grids [(p0, d1, r1, d2, r2)]."""
    if not pts:
        return []
    chains = _greedy_chains_nd(pts)
    byshape = defaultdict(list)
    for p0, d, r in chains:
        byshape[(d, r)].append(p0)
    grids = []
    for (d, r), bases in sorted(byshape.items()):
        for b0, d2, r2 in _greedy_chains_nd(bases):
            grids.append((b0, d, r, d2, r2))
    return grids


def _make_plan(a, b, o, c):
    T = len(a)
    cr = np.round(np.asarray(c, dtype=np.float64), 10)
    by_o = defaultdict(list)
    for i in range(T):
        by_o[int(o[i])].append((int(a[i]), int(b[i]), float(cr[i])))
    outputs = sorted(by_o)
    n_out = len(outputs)
    assert outputs == list(range(n_out))

    prod_consumers = defaultdict(list)
    for oo in outputs:
        for (ai, bi, ci) in by_o[oo]:
            prod_consumers[(ai, bi)].append(oo)

    def signature(oo):
        lst = by_o[oo]
        a0, b0, _ = lst[0]
        return (len(lst), tuple((ai - a0, bi - b0, round(ci, 8))
                                for ai, bi, ci in lst))
    classes = defaultdict(list)
    for oo in outputs:
        classes[signature(oo)].append(oo)
    class_order = sorted(classes, key=lambda sg: classes[sg][0])
    cls_idx = {sg: i for i, sg in enumerate(class_order)}
    cls_of_out = {}
    for sg, olist in classes.items():
        for oo in olist:
            cls_of_out[oo] = cls_idx[sg]

    z_slot = {}
    w_specs = []           # (z_src, ratio) per w slot
    ws2_specs = []         # (z_src, ratio, o): scaled copy straight into s2
    direct_prods = []      # (a, b, o, sign)
    z_prods_pts = []       # (a, b, zslot) in assignment order
    out_terms = {}         # o -> list of ("z"/"w", idx, sign); first sign=+1
    out_scale = {}         # o -> final in-place scale on s2 (1.0 = skip)

    z_cls = {}

    def get_z(ai, bi, ci_):
        if (ai, bi) not in z_slot:
            z_slot[(ai, bi)] = len(z_slot)
            z_prods_pts.append((ai, bi, z_slot[(ai, bi)]))
            z_cls[z_slot[(ai, bi)]] = ci_
        return z_slot[(ai, bi)]

    for sg in class_order:
        for oo in classes[sg]:
            lst = by_o[oo]
            k = len(lst)
            if k == 1:
                ai, bi, ci = lst[0]
                excl = len(prod_consumers[(ai, bi)]) == 1
                if excl and abs(abs(ci) - 1.0) < 1e-9:
                    direct_prods.append((ai, bi, oo, 1 if ci > 0 else -1))
                    out_terms[oo] = []
                    out_scale[oo] = 1.0
                else:
                    zi = get_z(ai, bi, cls_idx[sg])
                    ws2_specs.append((zi, float(ci), oo))
                    out_terms[oo] = []
                    out_scale[oo] = 1.0
                continue
            groups = defaultdict(list)
            for (ai, bi, ci) in lst:
                groups[round(abs(ci), 10)].append((ai, bi, ci))
            pivot_abs = max(groups, key=lambda g: (len(groups[g]), g))
            if len(groups[pivot_abs]) == 1:
                terms = []
                for (ai, bi, ci) in lst:
                    zi = get_z(ai, bi, cls_idx[sg])
                    w_specs.append((zi, float(ci)))
                    terms.append(("w", len(w_specs) - 1, 1))
                out_terms[oo] = terms
                out_scale[oo] = 1.0
            else:
                pivot = groups[pivot_abs]
                nonpivot = [x for g, mem in groups.items() if g != pivot_abs
                            for x in mem]
                first_sign = 1.0 if pivot[0][2] > 0 else -1.0
                scale = first_sign * pivot_abs
                terms = []
                for (ai, bi, ci) in pivot:
                    sgn = 1 if ci * first_sign > 0 else -1
                    terms.append(("z", get_z(ai, bi, cls_idx[sg]), sgn))
                for (ai, bi, ci) in nonpivot:
                    zi = get_z(ai, bi, cls_idx[sg])
                    w_specs.append((zi, float(ci / scale)))
                    terms.append(("w", len(w_specs) - 1, 1))
                out_terms[oo] = terms
                out_scale[oo] = float(scale)

    n_z = len(z_slot)
    n_w = len(w_specs)

    # windows
    win_edges = np.asarray(WIN_EDGES, int)
    assert win_edges[-1] == n_out
    n_win = len(win_edges) - 1
    win_of = np.zeros(n_out, int)
    for wi in range(n_win):
        win_of[win_edges[wi]:win_edges[wi + 1]] = wi

    # consumer windows for z and w slots
    z_minwin = defaultdict(lambda: n_win)
    w_minwin = defaultdict(lambda: n_win)
    for oo, terms in out_terms.items():
        for sp, idx, _ in terms:
            if sp == "z":
                z_minwin[idx] = min(z_minwin[idx], win_of[oo])
            else:
                w_minwin[idx] = min(w_minwin[idx], win_of[oo])
    for (zs, ratio, oo) in ws2_specs:
        z_minwin[zs] = min(z_minwin[zs], win_of[oo])
    for wi, (zs, ratio) in enumerate(w_specs):
        z_minwin[zs] = min(z_minwin[zs], w_minwin[wi])

    # --- grids, each tagged (minwin, stage)
    def pts_of(p0, d1, r1, d2, r2):
        return [tuple(p0[k] + i * d1[k] + j * d2[k] for k in range(len(p0)))
                for j in range(r2) for i in range(r1)]

    ops = []   # (wave, stage, kind, payload)
    # waves: dependency-correct emission order. Each grid's wave is the max of
    # its own min-window and every producer grid's wave; within a wave the
    # stage number orders producers before consumers (zp 0 < dp 1 < ws 2 <
    # ws2 3 < add step j at 4+j < ss 8 < conv/store at window close).
    z_wave = {}
    zp_by_cls = defaultdict(list)
    for pt in z_prods_pts:
        zp_by_cls[z_cls[pt[2]]].append(pt)
    for _ci, pts in sorted(zp_by_cls.items()):
        for p0, d1, r1, d2, r2 in _grid_cover(pts):
            mw = min(z_minwin[zi] for (_, _, zi) in pts_of(p0, d1, r1, d2, r2))
            ops.append((mw, 0, "zp", (p0, d1, r1, d2, r2)))
            for (_, _, zi) in pts_of(p0, d1, r1, d2, r2):
                z_wave[zi] = mw
    dp_bywin = defaultdict(list)
    for ai, bi, oo, s in direct_prods:
        dp_bywin[(s, int(win_of[oo]))].append((ai, bi, oo))
    out_last_wave = defaultdict(int)
    for (sgn, wv), pts in sorted(dp_bywin.items()):
        for p0, d1, r1, d2, r2 in _grid_cover(pts):
            ops.append((wv, 1, "dp", (sgn, p0, d1, r1, d2, r2)))
            for (_, _, oo) in pts_of(p0, d1, r1, d2, r2):
                out_last_wave[oo] = max(out_last_wave[oo], wv)
    w_wave = {}
    byr = defaultdict(list)
    for wi, (zs, ratio) in enumerate(w_specs):
        byr[(round(ratio, 10), int(w_minwin[wi]))].append((zs, wi))
    for (ratio, wv), pts in sorted(byr.items()):
        for p0, d1, r1, d2, r2 in _grid_cover(pts):
            pl_ = pts_of(p0, d1, r1, d2, r2)
            wave = max([wv] + [z_wave[zs] for (zs, _) in pl_])
            ops.append((wave, 2, "ws", (ratio, p0, d1, r1, d2, r2)))
            for (_, wi) in pl_:
                w_wave[wi] = wave
    byr2 = defaultdict(list)
    for (zs, ratio, oo) in ws2_specs:
        byr2[round(ratio, 10)].append((zs, oo))
    for ratio, pts in sorted(byr2.items()):
        for p0, d1, r1, d2, r2 in _grid_cover(pts):
            pl_ = pts_of(p0, d1, r1, d2, r2)
            wave = max([min(int(win_of[oo]) for (_, oo) in pl_)]
                       + [z_wave[zs] for (zs, _) in pl_])
            ops.append((wave, 3, "ws2", (ratio, p0, d1, r1, d2, r2)))
            for (_, oo) in pl_:
                out_last_wave[oo] = max(out_last_wave[oo], wave)
    # whole-class Pool offload: deepest classes within the window band
    cls_addblocks = defaultdict(int)
    cls_minwin = defaultdict(lambda: 10**9)
    cls_maxwin = defaultdict(int)
    cls_nout = defaultdict(int)
    for oo, terms in out_terms.items():
        ci_ = cls_of_out[oo]
        cls_nout[ci_] += 1
        cls_minwin[ci_] = min(cls_minwin[ci_], int(win_of[oo]))
        cls_maxwin[ci_] = max(cls_maxwin[ci_], int(win_of[oo]))
        if terms:
            cls_addblocks[ci_] += len(terms) - 1
    pool_classes = set()
    budget = POOL_CLASS_BLOCKS
    cand = [ci_ for ci_ in cls_addblocks
            if cls_minwin[ci_] >= POOL_CLASS_MINWIN
            and cls_maxwin[ci_] <= POOL_CLASS_MAXWIN]
    for ci_ in sorted(cand,
                      key=lambda c_: -cls_addblocks[c_] / max(1, cls_nout[c_])):
        blk = cls_addblocks[ci_]
        if blk <= budget:
            pool_classes.add(ci_)
            budget -= blk
        if budget <= 0:
            break

    add_groups = defaultdict(list)
    for oo, terms in out_terms.items():
        if not terms:
            continue
        sp0, i0, _ = terms[0]
        sp1, i1, g1 = terms[1]
        add_groups[(cls_of_out[oo], 0, sp0 + sp1, g1)].append((i0, i1, oo))
        for j, (spj, ij, gj) in enumerate(terms[2:], start=2):
            add_groups[(cls_of_out[oo], j, "s" + spj, gj)].append((oo, ij, oo))

    def src_wave(space, idx):
        return z_wave.get(idx, 0) if space == "z" else w_wave.get(idx, 0)

    for key, pts in sorted(add_groups.items(), key=lambda kv: str(kv[0])):
        step = key[1]
        pool = key[0] in pool_classes
        akey = (step,) + key[2:]
        for p0, d1, r1, d2, r2 in _grid_cover(pts):
            pl_ = pts_of(p0, d1, r1, d2, r2)
            mw = min(int(win_of[pt[2]]) for pt in pl_)
            if step == 0:
                spaces = key[2]
                dep = max(max(src_wave(spaces[0], pt[0]),
                              src_wave(spaces[1], pt[1])) for pt in pl_)
            else:
                space = key[2][1]
                dep = max(max(out_last_wave[pt[0]], src_wave(space, pt[1]))
                          for pt in pl_)
            wave = max(mw, dep)
            ops.append((wave, 4 + step, "add",
                        (akey, p0, d1, r1, d2, r2, pool)))
            for pt in pl_:
                out_last_wave[pt[2]] = max(out_last_wave[pt[2]], wave)
    # s2 scales: after every add touching their outputs
    bys = defaultdict(list)
    for oo in outputs:
        sc = out_scale[oo]
        if abs(sc - 1.0) > 1e-12:
            pool = cls_of_out[oo] in pool_classes
            bys[(round(sc, 10), int(win_of[oo]), pool)].append((oo,))
    for (sc, wi, pool), pts in sorted(bys.items()):
        for p0, d1, r1, d2, r2 in _grid_cover(pts):
            pl_ = pts_of(p0, d1, r1, d2, r2)
            wave = max([wi] + [out_last_wave[oo] for (oo,) in pl_])
            ops.append((wave, 8, "ss", (sc, p0, d1, r1, d2, r2, pool)))

    ops.sort(key=lambda t: (t[0], t[1]))

    return dict(
        n_out=n_out, n_z=n_z, n_w=n_w, ops=ops,
        win_edges=win_edges, n_win=n_win,
    )


# ----------------------------------------------------------------- bass build
_nc_cache = {}


def _build_nc(plan, in_dim, out_dim, b_core):
    import concourse.bacc as bacc
    from concourse import mybir
    from concourse.tile import TileContext

    f32 = mybir.dt.float32
    fp16 = mybir.dt.float16
    S = b_core // 128
    assert S == 2, "layout assumes 2 batch subtiles per core"
    n_ablk = in_dim // 32
    n_out = plan["n_out"]
    n_z = max(plan["n_z"], 1)
    n_w = max(plan["n_w"], 1)
    win_edges = plan["win_edges"]
    n_win = plan["n_win"]
    mult = mybir.AluOpType.mult
    addop = mybir.AluOpType.add
    subop = mybir.AluOpType.subtract
    copyf = mybir.ActivationFunctionType.Copy

    nc = bacc.Bacc("TRN2", target_bir_lowering=False)
    x1 = nc.dram_tensor("x1", [b_core, in_dim], f32, kind="ExternalInput")
    x2 = nc.dram_tensor("x2", [b_core, in_dim], f32, kind="ExternalInput")
    y = nc.dram_tensor("y", [b_core, out_dim], f32, kind="ExternalOutput")

    with TileContext(nc) as tc:
        with (
            tc.tile_pool(name="pin", bufs=1) as pin,
            tc.tile_pool(name="pz", bufs=1) as pz,
            tc.tile_pool(name="pstg", bufs=3) as pstg,
        ):
            x1t = pin.tile([128, n_ablk * 64], fp16, tag="x1t")
            x1n = pin.tile([128, n_ablk * 64], fp16, tag="x1n")
            x2t = pin.tile([128, n_ablk * 64], fp16, tag="x2t")
            zt = pz.tile([128, n_z * 64], fp16, tag="zt")
            wt = pz.tile([128, n_w * 64], fp16, tag="wt")
            s2 = pz.tile([128, n_out * 64], fp16, tag="s2")

            # interleaved loads + cast/reorder: x1 on ACT, x2 on DVE
            lstg1 = pstg.tile([128, S, in_dim], f32, tag="lstg1")
            lstg2 = pstg.tile([128, S, in_dim], f32, tag="lstg2")
            xv1 = x1t[:].rearrange("p (f s c) -> p s f c", s=S, c=32)
            xv2 = x2t[:].rearrange("p (f s c) -> p s f c", s=S, c=32)
            for s in range(S):
                for xt, xv, xd, lstg, on_act in (
                        (x1t, xv1, x1, lstg1, True),
                        (x2t, xv2, x2, lstg2, False)):
                    nc.sync.dma_start(out=lstg[:, s, :],
                                      in_=xd[s * 128:(s + 1) * 128, :])
                    src_ap = lstg[:, s, :].rearrange("p (f c) -> p f c", c=32)
                    if on_act:
                        nc.scalar.activation(out=xv[:, s], in_=src_ap,
                                             func=copyf)
                    else:
                        nc.vector.tensor_copy(out=xv[:, s], in_=src_ap)
            nc.vector.tensor_scalar(out=x1n[:], in0=x1t[:], scalar1=-1.0,
                                    scalar2=None, op0=mult)

            flat = {
                "x1t": (x1t[:], n_ablk * 64), "x1n": (x1n[:], n_ablk * 64),
                "x2t": (x2t[:], n_ablk * 64), "z": (zt[:], n_z * 64),
                "w": (wt[:], n_w * 64), "s2": (s2[:], n_out * 64),
            }

            def gap(space, ci, grid):
                """AP for coordinate ci of grid (p0,d1,r1,d2,r2) over `space`."""
                base, tot = flat[space]
                p0, d1, r1, d2, r2 = grid
                dims = [[tot, 128]]
                if r2 > 1:
                    dims.append([d2[ci] * 64, r2])
                if r1 > 1:
                    dims.append([d1[ci] * 64, r1])
                dims.append([1, 64])
                return dataclasses.replace(
                    base, ap=dims, offset=p0[ci] * 64)

            # window close index: last op (by emission order) touching window w
            close_at = {}
            for idx, (mw, st, kind, pl) in enumerate(plan["ops"]):
                if kind in ("dp", "ws2", "add", "ss"):
                    if kind == "dp":
                        grid = pl[1:]
                        owins = [int(np.searchsorted(win_edges, oo, "right") - 1)
                                 for (_, _, oo) in _pts(grid)]
                    elif kind == "ws2":
                        grid = pl[1:]
                        owins = [int(np.searchsorted(win_edges, oo, "right") - 1)
                                 for (_, oo) in _pts(grid)]
                    elif kind == "add":
                        grid = pl[1:6]
                        owins = [int(np.searchsorted(win_edges, pt[2], "right") - 1)
                                 for pt in _pts(grid)]
                    else:
                        grid = pl[1:6]
                        owins = [int(np.searchsorted(win_edges, oo, "right") - 1)
                                 for (oo,) in _pts(grid)]
                    for wv in set(owins):
                        close_at[wv] = idx
            # windows with no ops at all (shouldn't happen) close at start
            for wv in range(n_win):
                close_at.setdefault(wv, -1)

            def emit_window(wv):
                o_lo, o_hi = int(win_edges[wv]), int(win_edges[wv + 1])
                run = o_hi - o_lo
                w32 = run * 32
                stg = pstg.tile([128, S, w32], f32, tag="stg")
                src = dataclasses.replace(
                    s2[:], ap=[[n_out * 64, 128], [64, run], [32, S], [1, 32]],
                    offset=o_lo * 64)
                dst = dataclasses.replace(
                    stg[:], ap=[[S * w32, 128], [32, run], [w32, S], [1, 32]],
                    offset=0)
                eng = CONV_ENG[wv % len(CONV_ENG)]
                if eng == "a":
                    nc.scalar.activation(out=dst, in_=src, func=copyf)
                elif eng == "p":
                    nc.gpsimd.tensor_copy(out=dst, in_=src)
                else:
                    nc.vector.tensor_copy(out=dst, in_=src)
                for s in range(S):
                    nc.sync.dma_start(
                        out=y[s * 128:(s + 1) * 128, o_lo * 32:o_hi * 32],
                        in_=stg[:, s, :],
                    )

            emitted_windows = set()
            for idx, (mw, st, kind, pl) in enumerate(plan["ops"]):
                if kind == "zp":
                    grid = pl
                    nc.vector.tensor_tensor(
                        out=gap("z", 2, grid), in0=gap("x1t", 0, grid),
                        in1=gap("x2t", 1, grid), op=mult)
                elif kind == "dp":
                    sgn, grid = pl[0], pl[1:]
                    src1 = "x1t" if sgn > 0 else "x1n"
                    nc.vector.tensor_tensor(
                        out=gap("s2", 2, grid), in0=gap(src1, 0, grid),
                        in1=gap("x2t", 1, grid), op=mult)
                elif kind == "ws":
                    ratio, grid = pl[0], pl[1:]
                    nc.vector.tensor_scalar(
                        out=gap("w", 1, grid), in0=gap("z", 0, grid),
                        scalar1=float(ratio), scalar2=None, op0=mult)
                elif kind == "ws2":
                    ratio, grid = pl[0], pl[1:]
                    nc.vector.tensor_scalar(
                        out=gap("s2", 1, grid), in0=gap("z", 0, grid),
                        scalar1=float(ratio), scalar2=None, op0=mult)
                elif kind == "add":
                    key, grid, pool = pl[0], pl[1:6], pl[6]
                    eng = nc.gpsimd if pool else nc.vector
                    if key[0] == 0:
                        spaces, sgn = key[1], key[2]
                        sp0 = "z" if spaces[0] == "z" else "w"
                        sp1 = "z" if spaces[1] == "z" else "w"
                        eng.tensor_tensor(
                            out=gap("s2", 2, grid), in0=gap(sp0, 0, grid),
                            in1=gap(sp1, 1, grid),
                            op=addop if sgn > 0 else subop)
                    else:
                        space, sgn = key[1][1], key[2]
                        spj = "z" if space == "z" else "w"
                        eng.tensor_tensor(
                            out=gap("s2", 2, grid), in0=gap("s2", 0, grid),
                            in1=gap(spj, 1, grid),
                            op=addop if sgn > 0 else subop)
                else:  # ss
                    sc, grid, pool = pl[0], pl[1:6], pl[6]
                    eng = nc.gpsimd if pool else nc.vector
                    eng.tensor_scalar(
                        out=gap("s2", 0, grid), in0=gap("s2", 0, grid),
                        scalar1=float(sc), scalar2=None, op0=mult)
                for wv in range(n_win):
                    if close_at.get(wv) == idx and wv not in emitted_windows:
                        emitted_windows.add(wv)
                        emit_window(wv)
            for wv in range(n_win):
                if wv not in emitted_windows:
                    emit_window(wv)
    nc.finalize()
    return nc


def _pts(grid):
    p0, d1, r1, d2, r2 = grid
    return [tuple(p0[k] + i * d1[k] + j * d2[k] for k in range(len(p0)))
            for j in range(r2) for i in range(r1)]


def _get_nc(triples, in_dim, out_dim, b_core):
    a, b, o, c = triples
    key = hash((in_dim, out_dim, b_core, tuple(a), tuple(b), tuple(o),
                tuple(np.asarray(c).tolist())))
    if key not in _nc_cache:
        plan = _make_plan(a, b, o, c)
        _nc_cache[key] = _build_nc(plan, in_dim, out_dim, b_core)
    return _nc_cache[key]


# ----------------------------------------------------------------- entry point
def kernel(x1, x2, cg_tilde, repids_in1, repids_in2, repids_out, out_dim):
    x1 = np.ascontiguousarray(np.asarray(x1, dtype=np.float32))
    x2 = np.ascontiguousarray(np.asarray(x2, dtype=np.float32))
    cg = np.asarray(cg_tilde, dtype=np.float32)
    r1 = np.asarray(repids_in1)
    r2 = np.asarray(repids_in2)
    ro = np.asarray(repids_out)
    odim = int(np.asarray(out_dim))

    B, in_dim = x1.shape
    triples = _extract_triples(r1, r2, ro, cg)
    usable = (
        triples is not None and B % N_CORES == 0
        and (B // N_CORES) == 256 and odim % 32 == 0 and in_dim % 32 == 0
        and odim // 32 == WIN_EDGES[-1]
    )
    if not usable:
        return _numpy_fallback(x1, x2, cg, r1, r2, ro, odim)

    from concourse.bass_utils import run_bass_kernel_spmd

    b_core = B // N_CORES
    try:
        nc = _get_nc(triples, in_dim, odim, b_core)
    except Exception:
        return _numpy_fallback(x1, x2, cg, r1, r2, ro, odim)

    in_maps = [
        {"x1": x1[i * b_core:(i + 1) * b_core],
         "x2": x2[i * b_core:(i + 1) * b_core]}
        for i in range(N_CORES)
    ]
    res = run_bass_kernel_spmd(nc, in_maps, core_ids=list(range(N_CORES)))
    out = np.empty((B, odim), dtype=np.float32)
    for i in range(N_CORES):
        out[i * b_core:(i + 1) * b_core] = res.results[i]["y"]
    return out
